# revision 1
# baseline (speedup 1.0000x reference)
"""SSIM loss on 8 Trainium2 NeuronCores — H-sharded with 5-row halo exchange.

Each core gets a [522, 4106] band (512 own rows + 5-row halos, W edge-padded),
computes the 5 Gaussian-windowed fields (mu1, mu2, E[x^2], E[y^2], E[xy]) via
separable 11x11 convolution, the SSIM map, and a partial sum. Host sums the 8
partials (the final mean all-reduce).
"""

import numpy as np

H = W = 4096
NCORES = 8
RPC = H // NCORES  # 512
PAD = 5
WIN = 11
SIGMA = 1.5
BAND = RPC + 2 * PAD  # 522
WP = W + 2 * PAD      # 4106
C1 = (0.01 * 255) ** 2
C2 = (0.03 * 255) ** 2

_PMAP = None


def _gauss1d():
    x = np.arange(WIN) - (WIN - 1) / 2.0
    g = np.exp(-(x * x) / (2.0 * SIGMA * SIGMA))
    return (g / g.sum()).astype(np.float32)


def _build_pmap():
    import jax
    import jax.numpy as jnp

    g = jnp.asarray(_gauss1d())

    def conv_sep(x):
        # x: [BAND, WP]; vertical then horizontal valid 1D convs -> [RPC, W]
        v = jnp.zeros((RPC, WP), jnp.float32)
        for k in range(WIN):
            v = v + g[k] * jax.lax.dynamic_slice(x, (k, 0), (RPC, WP))
        h = jnp.zeros((RPC, W), jnp.float32)
        for k in range(WIN):
            h = h + g[k] * jax.lax.dynamic_slice(v, (0, k), (RPC, W))
        return h

    def shard_fn(m1, t1, b1, m2, t2, b2):
        x1 = jnp.pad(jnp.concatenate([t1, m1, b1], 0), ((0, 0), (PAD, PAD)), mode="edge")
        x2 = jnp.pad(jnp.concatenate([t2, m2, b2], 0), ((0, 0), (PAD, PAD)), mode="edge")
        mu1 = conv_sep(x1)
        mu2 = conv_sep(x2)
        ex2 = conv_sep(x1 * x1)
        ey2 = conv_sep(x2 * x2)
        exy = conv_sep(x1 * x2)
        m12 = mu1 * mu2
        m1s = mu1 * mu1
        m2s = mu2 * mu2
        s1 = ex2 - m1s
        s2 = ey2 - m2s
        s12 = exy - m12
        num = (2 * m12 + C1) * (2 * s12 + C2)
        den = (m1s + m2s + C1) * (s1 + s2 + C2)
        return jnp.sum(num / den)

    return jax.pmap(shard_fn)


def kernel(img1: np.ndarray, img2: np.ndarray) -> np.ndarray:
    global _PMAP
    a = np.ascontiguousarray(np.asarray(img1, np.float32))
    b = np.ascontiguousarray(np.asarray(img2, np.float32))
    tidx = np.clip(RPC * np.arange(NCORES)[:, None] + np.arange(-PAD, 0)[None, :], 0, H - 1)
    bidx = np.clip(RPC * np.arange(NCORES)[:, None] + np.arange(RPC, RPC + PAD)[None, :], 0, H - 1)
    try:
        if _PMAP is None:
            _PMAP = _build_pmap()
        parts = np.asarray(
            _PMAP(a.reshape(NCORES, RPC, W), a[tidx], a[bidx],
                  b.reshape(NCORES, RPC, W), b[tidx], b[bidx]),
            np.float64,
        )
        return np.float32(parts.sum() / (H * W))
    except Exception:
        ap = np.pad(a, ((0, 0), (PAD, PAD)), mode="edge")
        bp = np.pad(b, ((0, 0), (PAD, PAD)), mode="edge")
        idx = np.clip(
            np.arange(-PAD, RPC + PAD)[None, :] + RPC * np.arange(NCORES)[:, None],
            0, H - 1,
        )
        return _numpy_ssim(ap[idx], bp[idx])


def _numpy_ssim(sa, sb):
    g = _gauss1d().astype(np.float64)

    def conv(x):  # x: [522, 4106] -> [512, 4096]
        v = np.zeros((RPC, WP))
        for k in range(WIN):
            v += g[k] * x[k:k + RPC, :]
        h = np.zeros((RPC, W))
        for k in range(WIN):
            h += g[k] * v[:, k:k + W]
        return h

    tot = 0.0
    for i in range(NCORES):
        x1 = sa[i].astype(np.float64)
        x2 = sb[i].astype(np.float64)
        mu1, mu2 = conv(x1), conv(x2)
        ex2, ey2, exy = conv(x1 * x1), conv(x2 * x2), conv(x1 * x2)
        m12, m1s, m2s = mu1 * mu2, mu1 * mu1, mu2 * mu2
        num = (2 * m12 + C1) * (2 * (exy - m12) + C2)
        den = (m1s + m2s + C1) * ((ex2 - m1s) + (ey2 - m2s) + C2)
        tot += float((num / den).sum())
    return np.float32(tot / (H * W))



# revision 2
# speedup vs baseline: 1.1058x; 1.1058x over previous
"""SSIM loss on 8 Trainium2 NeuronCores — Bass/Tile kernel, H-sharded.

Each core gets a [522, 4096] fp16 band (512 own rows + 5-row clamped halos).
On-device per core: separable 11x11 Gaussian conv of the 5 SSIM fields
(x1, x2, x1^2, x2^2, x1*x2) as banded matmuls on the PE array
(vertical conv -> PE-transpose -> horizontal conv in transposed space),
then the SSIM map + fused partial-sum reduction on DVE/ACT/GpSimd.
Output per core: [128, 33] fp32 partial sums; host sums / (H*W).

Column strips: 8 x 502 out-cols + 1 x 80 (padded widths 512 / 90) so each
PSUM tile fits exactly one 2 KiB bank. fp16 Gaussian taps are nudged so they
sum to exactly 1.0 (otherwise sigma^2 = E[x^2] - mu^2 inherits a (sum g)^2
mismatch that biases the SSIM mean by ~15%).
"""

import numpy as np

H = W = 4096
NCORES = 8
RPC = H // NCORES            # 512 own rows per core
PAD = 5
WIN = 11
SIGMA = 1.5
RB = RPC + 2 * PAD           # 522 band rows
NRT = RPC // 128             # 4 output row tiles
NXT = 5                      # band stored as 5 row tiles (last has 10 rows)
C1 = (0.01 * 255) ** 2
C2 = (0.03 * 255) ** 2

SW = 502                     # out cols per full strip
NFULL = 8
STRIPS = [(s * SW, SW, SW + 2 * PAD) for s in range(NFULL)]
STRIPS.append((NFULL * SW, W - NFULL * SW, W - NFULL * SW + 2 * PAD))


def _jts_of(out_w):
    o = 0
    while o < out_w:
        yield o, min(128, out_w - o)
        o += 128


NJT = sum(len(list(_jts_of(w))) for _, w, _ in STRIPS)  # 33

_NC = None          # compiled Bass module (built once)
_CONSTS = None      # (wa, wb, ident) numpy fp16


def _gauss1d():
    x = np.arange(WIN) - (WIN - 1) / 2.0
    g = np.exp(-(x * x) / (2.0 * SIGMA * SIGMA))
    return (g / g.sum()).astype(np.float64)


def _gauss1d_f16():
    """fp16 taps nudged so their exact sum is 1.0."""
    g16 = _gauss1d().astype(np.float16)
    for _ in range(200):
        d = 1.0 - g16.astype(np.float64).sum()
        if abs(d) < 1e-9:
            break
        best, bi = None, None
        for i in range(len(g16)):
            nxt = np.nextafter(g16[i], np.float16(np.inf if d > 0 else -np.inf))
            sf = float(nxt) - float(g16[i])
            if abs(sf) <= abs(d) * 1.0000001 and (best is None or abs(sf) > abs(best)):
                best, bi = sf, i
        if bi is None:
            break
        g16[bi] = np.nextafter(g16[bi], np.float16(np.inf if d > 0 else -np.inf))
    return g16.astype(np.float64)


def _band_weights():
    """Wa [128,128], Wb [10,128]: W[k, m] = g[k - m] (vertical & horizontal)."""
    g = _gauss1d_f16()
    wa = np.zeros((128, 128), np.float64)
    wb = np.zeros((10, 128), np.float64)
    for m in range(128):
        for k in range(m, min(m + WIN, 128)):
            wa[k, m] = g[k - m]
        for k in range(128, m + WIN):
            wb[k - 128, m] = g[k - m]
    return wa.astype(np.float16), wb.astype(np.float16)


def _build_nc():
    import concourse.mybir as mybir
    import concourse.tile as tile
    from concourse import bacc

    F16 = mybir.dt.float16
    F32 = mybir.dt.float32
    AX = mybir.AluOpType
    SQF = mybir.ActivationFunctionType.Square
    CPF = mybir.ActivationFunctionType.Copy

    nc = bacc.Bacc(trn_type="TRN2")
    x1 = nc.dram_tensor("x1", [RB, W], F16, kind="ExternalInput")
    x2 = nc.dram_tensor("x2", [RB, W], F16, kind="ExternalInput")
    wad = nc.dram_tensor("wa", [128, 128], F16, kind="ExternalInput")
    wbd = nc.dram_tensor("wb", [10, 128], F16, kind="ExternalInput")
    idd = nc.dram_tensor("ident", [128, 128], F16, kind="ExternalInput")
    part = nc.dram_tensor("part", [128, NJT], F32, kind="ExternalOutput")

    with tile.TileContext(nc) as tc:
        with (
            tc.tile_pool(name="persist", bufs=1) as pp,
            tc.tile_pool(name="xs", bufs=2) as sp,
            tc.tile_pool(name="sq", bufs=6) as sqp,
            tc.tile_pool(name="vsb", bufs=3) as vp,
            tc.tile_pool(name="vt", bufs=2) as vtp,
            tc.tile_pool(name="map", bufs=2) as mp,
            tc.tile_pool(name="vps", bufs=2, space="PSUM") as vpsp,
            tc.tile_pool(name="tps", bufs=2, space="PSUM") as tpsp,
            tc.tile_pool(name="hps", bufs=4, space="PSUM") as hpsp,
        ):
            wa = pp.tile([128, 128], F16, tag="wa")
            wb = pp.tile([10, 128], F16, tag="wb")
            ident = pp.tile([128, 128], F16, tag="ident")
            nc.sync.dma_start(out=wa[:, :], in_=wad[:, :])
            nc.sync.dma_start(out=wb[:, :], in_=wbd[:, :])
            nc.sync.dma_start(out=ident[:, :], in_=idd[:, :])

            acc = pp.tile([128, NJT], F32, tag="acc")
            nc.vector.memset(acc[:, :], 0.0)

            jcol = 0
            for s, (oc0, ow, pw) in enumerate(STRIPS):
                # padded p in [oc0, oc0+pw); unpadded u = clamp(p-5, 0, W-1)
                ulo = max(oc0 - PAD, 0)
                uhi = min(oc0 - PAD + pw, W)
                un = uhi - ulo
                bc_lo = (oc0 - PAD) < 0
                bc_hi = (oc0 - PAD + pw) > W

                xt = {}
                for img, xd in ((1, x1), (2, x2)):
                    for rt in range(NXT):
                        rows = 128 if rt < 4 else RB - 512
                        t = sp.tile([128, un], F16, tag=f"x{img}_{rt}")
                        nc.sync.dma_start(
                            out=t[:rows, :],
                            in_=xd[rt * 128 : rt * 128 + rows, ulo:uhi],
                        )
                        xt[(img, rt)] = t

                sq = {}
                for rt in range(NXT):
                    rows = 128 if rt < 4 else RB - 512
                    a = xt[(1, rt)][:rows, :]
                    b = xt[(2, rt)][:rows, :]
                    t11 = sqp.tile([128, un], F16, tag="sq11")
                    t22 = sqp.tile([128, un], F16, tag="sq22")
                    t12 = sqp.tile([128, un], F16, tag="sq12")
                    nc.vector.tensor_mul(t11[:rows, :], a, a)
                    nc.vector.tensor_mul(t22[:rows, :], b, b)
                    nc.vector.tensor_mul(t12[:rows, :], a, b)
                    # field order: f=2 -> exy (t12), f=3 -> ex2 (t11), f=4 -> ey2
                    sq[rt] = (t12, t11, t22)

                def rhs_piece(f, rt, rows):
                    """(vcol0, width, ap) pieces covering padded cols [0, pw)."""
                    if f == 0:
                        src = xt[(1, rt)]
                    elif f == 1:
                        src = xt[(2, rt)]
                    else:
                        src = sq[rt][f - 2]
                    pieces = []
                    off = 0
                    if bc_lo:
                        pieces.append(
                            (0, PAD, src[:rows, 0:1].to_broadcast([rows, PAD]))
                        )
                        off = PAD
                    pieces.append((off, un, src[:rows, 0:un]))
                    if bc_hi:
                        pieces.append(
                            (off + un, PAD,
                             src[:rows, un - 1 : un].to_broadcast([rows, PAD]))
                        )
                    return pieces

                nct = (pw + 127) // 128
                vtt = {}
                for f in range(5):
                    vsb = {}
                    for rt in range(NRT):
                        vps = vpsp.tile([128, pw], F32, tag="vps")
                        piecesA = rhs_piece(f, rt, 128)
                        piecesB = rhs_piece(f, rt + 1, 10)
                        for (vc0, vw, apA), (_, _, apB) in zip(piecesA, piecesB):
                            nc.tensor.matmul(
                                vps[:, vc0 : vc0 + vw], wa[:, :], apA,
                                start=True, stop=False,
                            )
                            nc.tensor.matmul(
                                vps[:, vc0 : vc0 + vw], wb[:, :], apB,
                                start=False, stop=True,
                            )
                        vs = vp.tile([128, pw], F16, tag=f"vsb{rt}")
                        nc.scalar.activation(vs[:, :], vps[:, :], CPF)
                        vsb[rt] = vs
                    for ct in range(nct):
                        cw = min(128, pw - ct * 128)
                        tps = tpsp.tile([128, RPC], F16, tag="tps")
                        for rt in range(NRT):
                            nc.tensor.matmul(
                                tps[:cw, rt * 128 : (rt + 1) * 128],
                                vsb[rt][:, ct * 128 : ct * 128 + cw],
                                ident[:, :],
                                is_transpose=True, start=True, stop=True,
                            )
                        vt = vtp.tile([128, RPC], F16, tag=f"vt{f}_{ct}")
                        if ct % 2 == 0:
                            nc.scalar.activation(vt[:cw, :], tps[:cw, :], CPF)
                        else:
                            nc.vector.tensor_copy(vt[:cw, :], tps[:cw, :])
                        vtt[(f, ct)] = vt

                for (jo, jw) in _jts_of(ow):
                    jt = jo // 128

                    def hconv(f):
                        hp = hpsp.tile([128, RPC], F32, tag="hps")
                        cwA = min(128, pw - jt * 128)
                        needB = jw + WIN - 1 > cwA
                        nc.tensor.matmul(
                            hp[:jw, :], wa[:cwA, :jw], vtt[(f, jt)][:cwA, :],
                            start=True, stop=not needB,
                        )
                        if needB:
                            nc.tensor.matmul(
                                hp[:jw, :], wb[:, :jw], vtt[(f, jt + 1)][:10, :],
                                start=False, stop=True,
                            )
                        return hp

                    h_mu1 = hconv(0)
                    h_mu2 = hconv(1)
                    m1s = mp.tile([128, RPC], F32, tag="m1s")
                    m2s = mp.tile([128, RPC], F32, tag="m2s")
                    mu2s = mp.tile([128, RPC], F32, tag="mu2s")
                    m12 = mp.tile([128, RPC], F32, tag="m12")
                    nc.scalar.activation(m1s[:jw, :], h_mu1[:jw, :], SQF)
                    nc.scalar.activation(mu2s[:jw, :], h_mu2[:jw, :], CPF)
                    nc.vector.tensor_mul(m12[:jw, :], h_mu1[:jw, :], mu2s[:jw, :])
                    nc.scalar.activation(m2s[:jw, :], h_mu2[:jw, :], SQF)

                    A = mp.tile([128, RPC], F32, tag="A")
                    m2n = mp.tile([128, RPC], F32, tag="m2n")
                    nc.vector.tensor_scalar(
                        A[:jw, :], m12[:jw, :], 2.0, C1, op0=AX.mult, op1=AX.add
                    )
                    nc.vector.tensor_scalar(
                        m2n[:jw, :], m12[:jw, :], -2.0, C2, op0=AX.mult, op1=AX.add
                    )
                    h_exy = hconv(2)
                    B = mp.tile([128, RPC], mybir.dt.float16, tag="B")
                    nc.vector.scalar_tensor_tensor(
                        B[:jw, :], h_exy[:jw, :], 2.0, m2n[:jw, :],
                        op0=AX.mult, op1=AX.add,
                    )
                    num = mp.tile([128, RPC], F32, tag="num")
                    nc.vector.tensor_mul(num[:jw, :], A[:jw, :], B[:jw, :])

                    h_ex2 = hconv(3)
                    s1 = mp.tile([128, RPC], F32, tag="s1")
                    nc.vector.tensor_sub(s1[:jw, :], h_ex2[:jw, :], m1s[:jw, :])
                    h_ey2 = hconv(4)
                    s2 = mp.tile([128, RPC], F32, tag="s2")
                    nc.vector.tensor_sub(s2[:jw, :], h_ey2[:jw, :], m2s[:jw, :])

                    Q = mp.tile([128, RPC], F32, tag="Q")
                    nc.gpsimd.tensor_add(Q[:jw, :], m1s[:jw, :], m2s[:jw, :])
                    D = mp.tile([128, RPC], F32, tag="D")
                    nc.vector.scalar_tensor_tensor(
                        D[:jw, :], s1[:jw, :], C2, s2[:jw, :],
                        op0=AX.add, op1=AX.add,
                    )
                    den = mp.tile([128, RPC], F32, tag="den")
                    nc.vector.scalar_tensor_tensor(
                        den[:jw, :], Q[:jw, :], C1, D[:jw, :],
                        op0=AX.add, op1=AX.mult,
                    )
                    rec = mp.tile([128, RPC], F32, tag="rec")
                    nc.vector.reciprocal_approx_fast(rec[:jw, :], den[:jw, :])
                    sc = mp.tile([128, RPC], F32, tag="sc")
                    nc.vector.scalar_tensor_tensor(
                        sc[:jw, :], num[:jw, :], 1.0, rec[:jw, :],
                        op0=AX.mult, op1=AX.mult,
                        accum_out=acc[:jw, jcol : jcol + 1],
                    )
                    jcol += 1

            nc.sync.dma_start(out=part[:, :], in_=acc[:, :])
    nc.compile()
    return nc


def _make_bands(a):
    """[H, W] fp16 -> [NCORES, RB, W] with clamped halo rows."""
    idx = np.clip(
        np.arange(-PAD, RPC + PAD)[None, :] + RPC * np.arange(NCORES)[:, None],
        0, H - 1,
    )
    return a[idx]


def _host_inputs(img1, img2):
    global _CONSTS
    if _CONSTS is None:
        wa, wb = _band_weights()
        ident = np.eye(128, dtype=np.float16)
        _CONSTS = (wa, wb, ident)
    wa, wb, ident = _CONSTS
    ab = _make_bands(img1.astype(np.float16))
    bb = _make_bands(img2.astype(np.float16))
    return [
        {"x1": ab[i], "x2": bb[i], "wa": wa, "wb": wb, "ident": ident}
        for i in range(NCORES)
    ]


def _run_bass(img1, img2):
    global _NC
    from concourse.bass_utils import run_bass_kernel_spmd

    if _NC is None:
        _NC = _build_nc()
    in_maps = _host_inputs(img1, img2)
    res = run_bass_kernel_spmd(_NC, in_maps, core_ids=list(range(NCORES)))
    tot = sum(r["part"].astype(np.float64).sum() for r in res.results)
    return np.float32(tot / (H * W))


def _numpy_ssim(img1, img2):
    g = _gauss1d()

    def conv(x):
        xp = np.pad(x, ((PAD, PAD), (PAD, PAD)), mode="edge")
        v = np.zeros((H, W + 2 * PAD))
        for k in range(WIN):
            v += g[k] * xp[k : k + H, :]
        h = np.zeros((H, W))
        for k in range(WIN):
            h += g[k] * v[:, k : k + W]
        return h

    a = img1.astype(np.float64)
    b = img2.astype(np.float64)
    mu1, mu2 = conv(a), conv(b)
    ex2, ey2, exy = conv(a * a), conv(b * b), conv(a * b)
    m12, m1s, m2s = mu1 * mu2, mu1 * mu1, mu2 * mu2
    num = (2 * m12 + C1) * (2 * (exy - m12) + C2)
    den = (m1s + m2s + C1) * ((ex2 - m1s) + (ey2 - m2s) + C2)
    return np.float32((num / den).mean())


def kernel(img1: np.ndarray, img2: np.ndarray) -> np.ndarray:
    a = np.ascontiguousarray(np.asarray(img1, np.float32))
    b = np.ascontiguousarray(np.asarray(img2, np.float32))
    try:
        return _run_bass(a, b)
    except Exception:
        return _numpy_ssim(a, b)
